# revision 2
# baseline (speedup 1.0000x reference)
"""GCN (2-layer, Kipf-Welling) forward on 8 Trainium2 NeuronCores.

Single fused launch, graph/data parallel by destination node:
  - Nodes are assigned to (core, block, part) cells balanced by degree.
  - Phase T: every core computes the full transformed table
    T1 = (x * dinv) @ W1  ([N,64] bf16, rows padded to 256B) on the
    TensorEngine from a host-pre-transposed xsT input.
  - Phase A (layer 1): per destination block, source rows are fetched
    on-device with gpsimd dma_gather (int16 indices, 4 table chunks of
    32768 rows), segment-summed on the TensorEngine via a one-hot
    selection matrix, then scaled (dinv^2), biased, ReLU'd into the
    core's hidden chunk H_c (dinv-scaled).
  - Halo exchange: AllGather of the 8 H_c chunks (HBM-HBM collective)
    builds the full hidden table H on every core.
  - Phase B (layer 2): same gather + segment-sum from H, epilogue dots
    with W2, scales by dinv and adds b2 -> per-dest output.
"""

import math

import numpy as np
import ml_dtypes

import concourse.bacc as bacc
import concourse.bass as bass
import concourse.mybir as mybir
import concourse.tile as tile
from concourse._compat import get_trn_type
from concourse.bass_utils import run_bass_kernel_spmd

P = 128
N_CORES = 8
IN_DIM = 128
HID = 64
ROW = 2 * HID          # table row elements (bf16; upper half garbage pad)
CHUNK = 32768          # gather index range (int16)
NCHUNK = 4
GT = 8                 # transform tiles per group
f32 = mybir.dt.float32
bf16 = mybir.dt.bfloat16
i16 = mybir.dt.int16

_COMPILE_CACHE = {}


# ---------------------------------------------------------------------------
# host-side preprocessing
# ---------------------------------------------------------------------------

def _schedule_streams(dst_cell, rel_of_edge, row_of_src, nblk):
    """Bucket edges by (core, block) cell, sort each cell's stream by table
    chunk, pad each (cell, chunk) segment to a multiple of 128 slots with a
    tile count uniform across cores.

    Returns (idx_w, rel_arr, seg_tiles, off_blk, TTOT):
      idx_w   [cores, 128, TTOT*8] int16  wrapped gather indices
      rel_arr [cores, 128, TTOT] float32  dest part id per slot (200 = pad)
      seg_tiles [nblk][NCHUNK] int        tiles per (block, chunk)
      off_blk [nblk] int                  tile offset of each block
    """
    n_cells = N_CORES * nblk
    chunk_of_edge = row_of_src // CHUNK
    key = dst_cell * NCHUNK + chunk_of_edge
    order = np.argsort(key, kind="stable")
    key_s = key[order]
    rel_s = rel_of_edge[order]
    local_s = (row_of_src - chunk_of_edge * CHUNK)[order].astype(np.int16)

    counts = np.bincount(key_s, minlength=n_cells * NCHUNK).reshape(
        N_CORES, nblk, NCHUNK
    )
    # uniform-across-cores tiles per (block, chunk)
    seg_tiles = np.ceil(counts.max(axis=0) / P).astype(np.int64)  # [nblk, NCHUNK]
    blk_tiles = seg_tiles.sum(axis=1)  # [nblk]
    off_blk = np.zeros(nblk, dtype=np.int64)
    np.cumsum(blk_tiles[:-1], out=off_blk[1:])
    TTOT = int(blk_tiles.sum())

    idx_full = np.zeros((N_CORES, TTOT * P), dtype=np.int16)
    rel_full = np.full((N_CORES, TTOT * P), 200.0, dtype=np.float32)

    # slot offset of each (core, blk, chunk) segment within the core stream
    seg_off = (off_blk[None, :, None] + np.concatenate(
        [np.zeros((nblk, 1), np.int64), np.cumsum(seg_tiles, axis=1)[:, :-1]],
        axis=1,
    )[None, :, :]) * P  # [1, nblk, NCHUNK]
    seg_off = np.broadcast_to(seg_off, (N_CORES, nblk, NCHUNK))

    starts = np.zeros(n_cells * NCHUNK + 1, dtype=np.int64)
    np.cumsum(counts.reshape(-1), out=starts[1:])
    within = np.arange(len(key_s)) - starts[key_s]
    e_core = key_s // (nblk * NCHUNK)
    e_blk = (key_s // NCHUNK) % nblk
    e_chunk = key_s % NCHUNK
    pos = seg_off[e_core, e_blk, e_chunk] + within
    idx_full[e_core, pos] = local_s
    rel_full[e_core, pos] = rel_s

    # wrap indices: slot g -> (partition g%16, pos g//16), replicated 8x
    idx_w = np.ascontiguousarray(
        np.tile(
            idx_full.reshape(N_CORES, TTOT * 8, 16).transpose(0, 2, 1),
            (1, 8, 1),
        )
    )  # [cores, 128, TTOT*8]
    # rel layout: [cores, P, TTOT] with slot (t*128+p) -> rel_arr[c, p, t]
    rel_arr = np.ascontiguousarray(
        rel_full.reshape(N_CORES, TTOT, P).transpose(0, 2, 1)
    )
    return idx_w, rel_arr, seg_tiles, off_blk, TTOT


def _preprocess(edge_index, n_nodes):
    dst = np.asarray(edge_index[0], dtype=np.int64)
    src = np.asarray(edge_index[1], dtype=np.int64)
    loops = np.arange(n_nodes, dtype=np.int64)
    dst = np.concatenate([dst, loops])
    src = np.concatenate([src, loops])

    deg = np.bincount(dst, minlength=n_nodes).astype(np.float64)
    dinv = np.where(deg > 0, 1.0 / np.sqrt(deg), 0.0).astype(np.float32)

    NBLK = math.ceil(n_nodes / (N_CORES * P))  # 98
    NSLOT = N_CORES * P * NBLK                 # 100352
    n_cells = N_CORES * NBLK

    # balanced dest -> (core, blk, part) assignment: snake by degree
    order = np.argsort(-deg, kind="stable")
    cells_fwd = np.arange(n_cells)
    node_cell = np.empty(n_nodes, dtype=np.int64)
    node_part = np.empty(n_nodes, dtype=np.int64)
    pos = 0
    rnd = 0
    while pos < n_nodes:
        take = min(n_cells, n_nodes - pos)
        cells = cells_fwd if (rnd % 2 == 0) else cells_fwd[::-1]
        node_cell[order[pos : pos + take]] = cells[:take]
        node_part[order[pos : pos + take]] = rnd
        pos += take
        rnd += 1
    assert rnd <= P
    node_core = node_cell // NBLK
    node_blk = node_cell % NBLK

    # table row maps
    # T1: transform-partition-major (node n at tile n//128, psum part n%128)
    n_tiles = NSLOT // P  # 784
    rowT1 = (loops % P) * n_tiles + loops // P
    # H: core-major slot order
    rowH = node_core * (P * NBLK) + node_part * NBLK + node_blk

    dst_cell = node_cell[dst]
    rel_edge = node_part[dst].astype(np.float32)

    idx1, rel1, seg1, off1, TT1 = _schedule_streams(
        dst_cell, rel_edge, rowT1[src], NBLK
    )
    idx2, rel2, seg2, off2, TT2 = _schedule_streams(
        dst_cell, rel_edge, rowH[src], NBLK
    )

    dinv_d = np.zeros((N_CORES, P, NBLK), dtype=np.float32)
    dinv_d[node_core, node_part, node_blk] = dinv
    return dict(
        dinv=dinv,
        NBLK=NBLK,
        NSLOT=NSLOT,
        idx1=idx1,
        rel1=rel1,
        seg1=seg1,
        off1=off1,
        TT1=TT1,
        idx2=idx2,
        rel2=rel2,
        seg2=seg2,
        off2=off2,
        TT2=TT2,
        dinv_d=dinv_d,
        node_core=node_core,
        node_part=node_part,
        node_blk=node_blk,
    )


# ---------------------------------------------------------------------------
# device program
# ---------------------------------------------------------------------------

def _build(NBLK, NSLOT, seg1, off1, TT1, seg2, off2, TT2):
    n_tiles = NSLOT // P
    TB1 = int(max(seg1.sum(axis=1)))
    TB2 = int(max(seg2.sum(axis=1)))
    TB = max(TB1, TB2)
    nc = bacc.Bacc(get_trn_type() or "TRN2", debug=False)

    xsT = nc.dram_tensor("xsT", [P, NSLOT], bf16, kind="ExternalInput")
    w1 = nc.dram_tensor("w1", [IN_DIM, HID], bf16, kind="ExternalInput")
    idx1 = nc.dram_tensor("idx1", [P, TT1 * 8], i16, kind="ExternalInput")
    rel1 = nc.dram_tensor("rel1", [P, TT1], bf16, kind="ExternalInput")
    idx2 = nc.dram_tensor("idx2", [P, TT2 * 8], i16, kind="ExternalInput")
    rel2 = nc.dram_tensor("rel2", [P, TT2], bf16, kind="ExternalInput")
    iota = nc.dram_tensor("iota", [P, P], bf16, kind="ExternalInput")
    b1d = nc.dram_tensor("b1d", [P, NBLK, HID], f32, kind="ExternalInput")
    dinv2 = nc.dram_tensor("dinv2", [P, NBLK], f32, kind="ExternalInput")
    dinvd = nc.dram_tensor("dinvd", [P, NBLK], f32, kind="ExternalInput")
    w2r = nc.dram_tensor("w2r", [P, HID], f32, kind="ExternalInput")
    b2r = nc.dram_tensor("b2r", [P, 1], f32, kind="ExternalInput")
    outv = nc.dram_tensor("outv", [P, NBLK], f32, kind="ExternalOutput")

    with tile.TileContext(nc) as tc:
        with (
            tc.tile_pool(name="dram", bufs=1, space="DRAM") as dramp,
            tc.tile_pool(name="const", bufs=1) as constp,
            tc.tile_pool(name="xt", bufs=3) as xp,
            tc.tile_pool(name="st", bufs=3) as stp,
            tc.tile_pool(name="idx", bufs=3) as idxp,
            tc.tile_pool(name="msg", bufs=3) as msgp,
            tc.tile_pool(name="sb", bufs=3) as sp,
            tc.tile_pool(name="t1", bufs=3) as t1p,
            tc.tile_pool(name="stage", bufs=1) as stagep,
            tc.tile_pool(name="psT", bufs=2, space="PSUM") as psT,
            tc.tile_pool(name="psA", bufs=3, space="PSUM") as psA,
            tc.tile_pool(name="psD", bufs=3, space="PSUM") as psD,
        ):
            T1 = dramp.tile([NSLOT, ROW], bf16)
            hloc = dramp.tile([P * NBLK, ROW], bf16)
            H = dramp.tile([NSLOT, ROW], bf16)

            w1b = constp.tile([IN_DIM, HID], bf16)
            iotab = constp.tile([P, P], bf16)
            rel1b = constp.tile([P, TT1], bf16)
            rel2b = constp.tile([P, TT2], bf16)
            b1b = constp.tile([P, NBLK, HID], f32)
            dinv2b = constp.tile([P, NBLK], f32)
            dinvb = constp.tile([P, NBLK], f32)
            w2b = constp.tile([P, HID], f32)
            b2b = constp.tile([P, 1], f32)
            hstage = stagep.tile([P, NBLK, ROW], bf16)
            ostage = stagep.tile([P, NBLK], f32)
            nc.sync.dma_start(w1b[:], w1[:])
            nc.sync.dma_start(iotab[:], iota[:])
            nc.sync.dma_start(rel1b[:], rel1[:])
            nc.sync.dma_start(rel2b[:], rel2[:])
            nc.sync.dma_start(b1b[:], b1d[:])
            nc.sync.dma_start(dinv2b[:], dinv2[:])
            nc.sync.dma_start(dinvb[:], dinvd[:])
            nc.sync.dma_start(w2b[:], w2r[:])
            nc.sync.dma_start(b2b[:], b2r[:])

            # ---- phase T: T1 = xs @ W1, rows in partition-major order
            T1v = T1[:].rearrange("(j i) f -> j i f", j=P)  # [128, n_tiles, ROW]
            for i0 in range(0, n_tiles, GT):
                xt = xp.tile([P, GT, P], bf16, tag="xt")
                nc.sync.dma_start(
                    xt[:],
                    xsT[:, i0 * P : (i0 + GT) * P].rearrange(
                        "f (g n) -> f g n", g=GT
                    ),
                )
                pt = psT.tile([P, GT * HID], f32, tag="pt")
                for g in range(GT):
                    nc.tensor.matmul(
                        pt[:, g * HID : (g + 1) * HID],
                        xt[:, g, :],
                        w1b[:],
                        start=True,
                        stop=True,
                    )
                st = stp.tile([P, GT, ROW], bf16, tag="st")
                nc.vector.tensor_copy(
                    out=st[:, :, 0:HID],
                    in_=pt[:].rearrange("p (g h) -> p g h", g=GT),
                )
                nc.sync.dma_start(T1v[:, i0 : i0 + GT, :], st[:])

            # ---- aggregation phases
            def aggregate(layer, b, table, seg_b, off_b, relb, idxd, epi):
                idxb = idxp.tile([P, TB * 8], i16, tag="idx")
                nc.sync.dma_start(
                    idxb[:, : int(seg_b.sum()) * 8],
                    idxd[:, off_b * 8 : (off_b + int(seg_b.sum())) * 8],
                )
                mb = msgp.tile([P, TB, ROW], bf16, tag="msg")
                t0 = 0
                for c in range(NCHUNK):
                    nt = int(seg_b[c])
                    if nt == 0:
                        continue
                    lo = c * CHUNK
                    hi = min((c + 1) * CHUNK, NSLOT)
                    nc.gpsimd.dma_gather(
                        out_ap=mb[:, t0 : t0 + nt, :],
                        in_ap=table[lo:hi, :],
                        idxs_ap=idxb[:, t0 * 8 : (t0 + nt) * 8],
                        num_idxs=nt * P,
                        num_idxs_reg=nt * P,
                        elem_size=ROW,
                        single_packet=False,
                    )
                    t0 += nt
                nb = t0
                sb = sp.tile([P, TB, P], bf16, tag="s")
                nc.vector.tensor_tensor(
                    out=sb[:, :nb, :],
                    in0=relb[:, off_b : off_b + nb, None].to_broadcast(
                        [P, nb, P]
                    ),
                    in1=iotab[:, None, :].to_broadcast([P, nb, P]),
                    op=mybir.AluOpType.is_equal,
                )
                ps = (psA if layer == 1 else psD).tile(
                    [P, HID], f32, tag=f"pa{layer}"
                )
                for t in range(nb):
                    nc.tensor.matmul(
                        ps[:],
                        sb[:, t, :],
                        mb[:, t, 0:HID],
                        start=(t == 0),
                        stop=(t == nb - 1),
                    )
                epi(b, ps)

            # layer 1
            def epi1(b, ps):
                t1 = t1p.tile([P, HID], f32, tag="t1")
                nc.vector.scalar_tensor_tensor(
                    out=t1[:],
                    in0=ps[:],
                    scalar=dinv2b[:, b : b + 1],
                    in1=b1b[:, b, :],
                    op0=mybir.AluOpType.mult,
                    op1=mybir.AluOpType.add,
                )
                nc.scalar.activation(
                    out=hstage[:, b, 0:HID],
                    in_=t1[:],
                    func=mybir.ActivationFunctionType.Relu,
                )

            for b in range(NBLK):
                aggregate(1, b, T1, seg1[b], int(off1[b]), rel1b, idx1, epi1)

            # exchange: hloc rows = part*NBLK + blk (partition-major)
            nc.sync.dma_start(
                hloc[:].rearrange("(p b) f -> p b f", p=P), hstage[:]
            )
            nc.gpsimd.collective_compute(
                "AllGather",
                mybir.AluOpType.bypass,
                replica_groups=[list(range(N_CORES))],
                ins=[hloc.opt()],
                outs=[H.opt()],
            )

            # layer 2
            def epi2(b, ps):
                scr = t1p.tile([P, HID], f32, tag="scr")
                acc = t1p.tile([P, 1], f32, tag="acc")
                nc.vector.tensor_tensor(
                    out=scr[:], in0=ps[:], in1=w2b[:], op=mybir.AluOpType.mult
                )
                nc.vector.tensor_reduce(
                    out=acc[:],
                    in_=scr[:],
                    axis=mybir.AxisListType.X,
                    op=mybir.AluOpType.add,
                )
                nc.vector.tensor_scalar(
                    out=ostage[:, b : b + 1],
                    in0=acc[:],
                    scalar1=dinvb[:, b : b + 1],
                    scalar2=b2b[:],
                    op0=mybir.AluOpType.mult,
                    op1=mybir.AluOpType.add,
                )

            for b in range(NBLK):
                aggregate(2, b, H, seg2[b], int(off2[b]), rel2b, idx2, epi2)

            nc.sync.dma_start(outv[:], ostage[:])
    nc.compile()
    return nc


# ---------------------------------------------------------------------------
# kernel entry
# ---------------------------------------------------------------------------

def _make_inputs(pp, x, W1, b1, W2, b2):
    dinv = pp["dinv"]
    NBLK, NSLOT = pp["NBLK"], pp["NSLOT"]
    n_nodes = x.shape[0]

    xs = (np.asarray(x, np.float32) * dinv[:, None]).astype(ml_dtypes.bfloat16)
    xsT = np.zeros((P, NSLOT), dtype=ml_dtypes.bfloat16)
    xsT[:, :n_nodes] = xs.T  # node n at column n = tile n//128 ... wait

    iota_np = np.tile(np.arange(P, dtype=np.float32)[None], (P, 1)).astype(
        ml_dtypes.bfloat16
    )
    dinv_d = pp["dinv_d"]
    b1d_np = dinv_d[:, :, :, None] * np.asarray(b1, np.float32)[None, None, None, :]
    dinv2_np = dinv_d * dinv_d
    w2r_np = np.tile(np.asarray(W2, np.float32)[:, 0][None, :], (P, 1))
    b2r_np = np.full((P, 1), float(np.asarray(b2)[0]), dtype=np.float32)

    in_maps = []
    for c in range(N_CORES):
        in_maps.append(
            {
                "xsT": xsT,
                "w1": np.asarray(W1, np.float32).astype(ml_dtypes.bfloat16),
                "idx1": pp["idx1"][c],
                "rel1": pp["rel1"][c].astype(ml_dtypes.bfloat16),
                "idx2": pp["idx2"][c],
                "rel2": pp["rel2"][c].astype(ml_dtypes.bfloat16),
                "iota": iota_np,
                "b1d": b1d_np[c].astype(np.float32),
                "dinv2": dinv2_np[c].astype(np.float32),
                "dinvd": dinv_d[c].astype(np.float32),
                "w2r": w2r_np.astype(np.float32),
                "b2r": b2r_np,
            }
        )
    return in_maps


def kernel(x, W1, b1, W2, b2, edge_index):
    x = np.asarray(x)
    n_nodes = x.shape[0]
    pp = _preprocess(np.asarray(edge_index), n_nodes)

    key = (n_nodes, pp["TT1"], pp["TT2"])
    if key not in _COMPILE_CACHE:
        import time as _t

        t0 = _t.time()
        nc = _build(
            pp["NBLK"], pp["NSLOT"], pp["seg1"], pp["off1"], pp["TT1"],
            pp["seg2"], pp["off2"], pp["TT2"],
        )
        print(f"[kernel] built in {_t.time()-t0:.1f}s", flush=True)
        _COMPILE_CACHE[key] = nc
    nc = _COMPILE_CACHE[key]

    in_maps = _make_inputs(pp, x, W1, b1, W2, b2)
    res = run_bass_kernel_spmd(nc, in_maps, core_ids=list(range(N_CORES))).results

    out = np.zeros(n_nodes, dtype=np.float32)
    nci, npi, nbi = pp["node_core"], pp["node_part"], pp["node_blk"]
    for c in range(N_CORES):
        ov = np.asarray(res[c]["outv"])  # [P, NBLK]
        sel = nci == c
        out[sel] = ov[npi[sel], nbi[sel]]
    return out
